# revision 1
# baseline (speedup 1.0000x reference)
"""Category-specific linear (MoE-routing style) Trainium2 Bass kernel.

Computes out[n] = x[n] @ W[cat_ids[n]] + b[cat_ids[n]] for
x: [N, M, D_IN] f32, cat_ids: [N] int64, W: [C, D_IN, D_H] f32, b: [C, D_H] f32.

Strategy (8-core SPMD, full inputs in / full output out):
  Host: stable-sort samples by category, split into 8 equal shards of
  N/8 samples (perfect load balance).  Within a shard, each category is a
  contiguous run; runs are padded to whole 128-row tiles (8 samples) so the
  device program is fully static.  x rows are pre-transposed on the host
  (fp32 has no DMA-transpose path on TRN2) into [2, 128, NT*128] so the
  contraction dim lands on SBUF partitions.  Each core also gets a small
  deduplicated weight table (its <=KMAX distinct categories) and a per-tile
  weight-slot index.
  Device: the weight table lives in SBUF; for each 128-row tile the weight
  slot index is loaded into a PE register (values_load) and the matmul's
  moving operand is selected with a dynamic slice - zero weight duplication
  in HBM traffic, no dynamic control flow.  Two accumulating matmuls per
  tile (contraction 256 = 2x128), PSUM -> SBUF copy, batched stores.
"""

import os
import sys

import numpy as np

for _p in ("/opt/trn_rl_repo",):
    if os.path.isdir(_p) and _p not in sys.path:
        sys.path.insert(0, _p)

import concourse.bass as bass  # noqa: E402
import concourse.mybir as mybir  # noqa: E402
import concourse.tile as tile  # noqa: E402
from concourse import bacc  # noqa: E402
from concourse.bass import ds  # noqa: E402
from concourse.bass_utils import run_bass_kernel_spmd  # noqa: E402

NCORES = 8
P = 128  # SBUF partitions / rows per tile
D_IN = 256  # contraction dim (2 chunks of 128)
D_H = 256  # output dim
ROWS_PER_SAMPLE = 16
SPT = P // ROWS_PER_SAMPLE  # samples per tile = 8
TB = 8  # tile-count quantum (NT is padded to a multiple of this)
TBI = 16  # tiles per index-register load
OB = 4  # tiles per psum group / DVE copy
OS = 8  # tiles per out-store DMA

# filled by kernel() for test harness introspection
last_results = None


def _pack(x, cat_ids, W):
    """Host-side routing: sort, shard, pad, transpose, dedup weights.

    Returns (in_maps, scatter_info, NT, KMAX).
    scatter_info[k] = (sample_ids_per_padded_slot [NT*SPT] int64, valid mask)
    """
    N, M, Din = x.shape
    assert M == ROWS_PER_SAMPLE and Din == D_IN
    assert N % NCORES == 0
    S = N // NCORES

    cat = np.asarray(cat_ids).astype(np.int64).ravel()
    order = np.argsort(cat, kind="stable")
    cats_sorted = cat[order]

    # global category runs over the sorted sample list
    bounds = np.flatnonzero(np.diff(cats_sorted)) + 1
    seg_starts = np.concatenate([[0], bounds])
    seg_ends = np.concatenate([bounds, [N]])
    segments = [
        (int(cats_sorted[s]), int(s), int(e))
        for s, e in zip(seg_starts, seg_ends)
    ]

    def pack(T):
        """Greedy-pack category runs into cores of <= T tiles each.

        A run cut mid-category always cuts at an SPT-sample multiple, so
        cuts cost no padding; only each core-local run tail pads to a tile.
        Returns (padded_ids, tile_cats) per core or None if > NCORES cores.
        """
        cores = []
        cur_ids, cur_tcats, used = [], [], 0
        rem = list(segments)
        i = 0

        def close():
            nonlocal cur_ids, cur_tcats, used
            cores.append((cur_ids, cur_tcats))
            cur_ids, cur_tcats, used = [], [], 0

        while i < len(rem):
            c, s, e = rem[i]
            n = e - s
            tiles_need = (n + SPT - 1) // SPT
            avail = T - used
            if avail >= tiles_need:
                npad = (-n) % SPT
                cur_ids.append(order[s:e])
                if npad:
                    cur_ids.append(np.full(npad, -1, np.int64))
                cur_tcats.extend([c] * tiles_need)
                used += tiles_need
                i += 1
            elif avail >= 1:
                take = avail * SPT  # n > take since tiles_need > avail
                cur_ids.append(order[s : s + take])
                cur_tcats.extend([c] * avail)
                used = T
                rem[i] = (c, s + take, e)
                close()
            else:
                close()
            if len(cores) > NCORES:
                return None
        if cur_tcats:
            close()
        if len(cores) > NCORES:
            return None
        while len(cores) < NCORES:
            cores.append(([], []))
        return cores

    lo, hi = (N // NCORES) // SPT, ((N // NCORES) // SPT) * 2 + 16
    while lo < hi:
        mid = (lo + hi) // 2
        if pack(mid) is not None:
            hi = mid
        else:
            lo = mid + 1
    NT = ((lo + 3) // 4) * 4  # multiple of OB
    cores = pack(NT)
    assert cores is not None

    # pad every core to NT tiles
    padded_ids = []
    tile_cats = []
    for k in range(NCORES):
        ids_parts, tcats = cores[k]
        n_have = len(tcats)
        extra = NT - n_have
        if extra:
            fill_cat = tcats[0] if tcats else 0
            tcats = tcats + [fill_cat] * extra
            ids_parts = ids_parts + [np.full(extra * SPT, -1, np.int64)]
        padded_ids.append(
            np.concatenate(ids_parts)
            if ids_parts
            else np.full(NT * SPT, -1, np.int64)
        )
        tile_cats.append(tcats)

    # per-core weight dedup
    uniq_list = []
    for k in range(NCORES):
        seen = dict()
        for c in tile_cats[k]:
            if c not in seen:
                seen[c] = len(seen)
        uniq_list.append(seen)
    KMAX = max(len(u) for u in uniq_list)

    np_in = _np_in_dtype()
    in_maps = []
    scatter = []
    for k in range(NCORES):
        ids = padded_ids[k]
        valid = ids >= 0
        # gather + zero-pad x rows: [NT*SPT, M, Din]
        Xr = np.zeros((NT * SPT, M, Din), np.float32)
        Xr[valid] = x[ids[valid]]
        # transpose to [Din, NT*P] then chunk the contraction dim
        xT = np.ascontiguousarray(
            Xr.reshape(NT * P, Din).T.astype(np_in)
        ).reshape(2, P, NT * P)

        seen = uniq_list[k]
        w_ids = list(seen.keys())
        w_ids += [w_ids[0]] * (KMAX - len(w_ids))
        Wp = W[np.asarray(w_ids, np.int64)]  # [KMAX, Din, D_H]
        Wl = np.ascontiguousarray(
            Wp.reshape(KMAX, 2, P, D_H).transpose(2, 1, 0, 3).astype(np_in)
        )  # [P, 2, KMAX, D_H]

        widx = np.asarray([seen[c] for c in tile_cats[k]], np.int32)[None, :]

        in_maps.append({"xT": xT, "Wl": Wl, "widx": widx})
        scatter.append((ids, valid))

    return in_maps, scatter, NT, KMAX


def _dt_mode():
    return os.environ.get("CSL_DT_MODE", "f16")


def _out_mode():
    return os.environ.get("CSL_OUT_DT", "f16")


def _np_in_dtype():
    import ml_dtypes

    return {
        "f16": np.float16,
        "bf16": ml_dtypes.bfloat16,
        "f32r": np.float32,
        "f32": np.float32,
    }[_dt_mode()]


def _mm_dt():
    return {
        "f16": mybir.dt.float16,
        "bf16": mybir.dt.bfloat16,
        "f32r": mybir.dt.float32r,
        "f32": mybir.dt.float32,
    }[_dt_mode()]


def _build(NT, KMAX):
    """Build the SPMD device program for NT tiles and KMAX weight slots."""
    mm_dt = _mm_dt()
    out_dt = mybir.dt.float32 if _out_mode() == "f32" else mybir.dt.float16
    f32 = mybir.dt.float32
    i32 = mybir.dt.int32
    static_idx = os.environ.get("CSL_STATIC", "0") == "1"

    nc = bacc.Bacc(
        "TRN2",
        target_bir_lowering=False,
        debug=False,
        enable_asserts=False,
        num_devices=NCORES,
    )
    NTR = NT * P
    GX = 16  # tiles per x-load DMA group
    xT_d = nc.dram_tensor("xT", [2, P, NTR], mm_dt, kind="ExternalInput").ap()
    W_d = nc.dram_tensor("Wl", [P, 2, KMAX, D_H], mm_dt, kind="ExternalInput").ap()
    wi_d = nc.dram_tensor("widx", [1, NT], i32, kind="ExternalInput").ap()
    # partition-major output layout: fully contiguous per-partition stores;
    # the host untransposes when scattering back
    out_d = nc.dram_tensor("out", [P, NT, D_H], out_dt, kind="ExternalOutput").ap()

    with tile.TileContext(nc) as tc:
        with (
            tc.tile_pool(name="wpool", bufs=1) as wpool,
            tc.tile_pool(name="xpool", bufs=4) as xpool,
            tc.tile_pool(name="opool", bufs=3) as opool,
            tc.tile_pool(name="psum", bufs=4, space="PSUM") as psum_pool,
        ):
            # widx first (tiny, unblocks index loads); W on the Scalar ring
            # so it issues in parallel with the Sync-ring x loads; the ic=0
            # half lands first so tile 0's first matmul can start sooner
            wi_sb = wpool.tile([1, NT], i32)
            nc.sync.dma_start(wi_sb[:], wi_d)
            W_sb = wpool.tile([P, 2, KMAX, D_H], mm_dt)
            nc.scalar.dma_start(W_sb[:, 0], W_d[:, 0])
            nc.scalar.dma_start(W_sb[:, 1], W_d[:, 1])

            for g0 in range(0, NT, GX):
                gx = min(GX, NT - g0)
                # loads on the Sync HWDGE ring; stores go on the Scalar ring
                # so a store waiting on DVE never blocks a prefetch load
                xt = xpool.tile([P, 2, GX * P], mm_dt)
                if g0 == 0:
                    # split the first group so the first tiles arrive early
                    h = gx // 2
                    nc.sync.dma_start(xt[:, 0, : h * P], xT_d[0, :, : h * P])
                    nc.sync.dma_start(xt[:, 1, : h * P], xT_d[1, :, : h * P])
                    nc.sync.dma_start(
                        xt[:, 0, h * P : gx * P], xT_d[0, :, h * P : gx * P]
                    )
                    nc.sync.dma_start(
                        xt[:, 1, h * P : gx * P], xT_d[1, :, h * P : gx * P]
                    )
                else:
                    nc.sync.dma_start(
                        xt[:, 0, : gx * P], xT_d[0, :, g0 * P : (g0 + gx) * P]
                    )
                    nc.sync.dma_start(
                        xt[:, 1, : gx * P], xT_d[1, :, g0 * P : (g0 + gx) * P]
                    )
                for i0 in range(0, gx, TBI):
                    ti = min(TBI, gx - i0)
                    if static_idx:
                        vals = (0,) * ti  # debug: no dynamic indexing
                    else:
                        # one TENSOR_LOAD for ti per-tile weight slots
                        _, vals = nc.values_load_multi_w_load_instructions(
                            wi_sb[0:1, g0 + i0 : g0 + i0 + ti],
                            engines=(mybir.EngineType.PE,),
                            min_val=0,
                            max_val=KMAX - 1,
                            skip_runtime_bounds_check=True,
                        )
                    for s0 in range(0, ti, OS):
                        os_ = min(OS, ti - s0)
                        ot = opool.tile([P, OS, D_H], out_dt)
                        for o0 in range(s0, s0 + os_, OB):
                            ob_ = min(OB, s0 + os_ - o0)
                            ps = psum_pool.tile([P, OB, D_H], f32)
                            for j in range(ob_):
                                tt = i0 + o0 + j  # tile within group
                                widx = vals[o0 + j]
                                nc.tensor.matmul(
                                    ps[:, j, :],
                                    xt[:, 0, tt * P : (tt + 1) * P],
                                    W_sb[:, 0, ds(widx, 1), :],
                                    start=True,
                                    stop=False,
                                )
                                nc.tensor.matmul(
                                    ps[:, j, :],
                                    xt[:, 1, tt * P : (tt + 1) * P],
                                    W_sb[:, 1, ds(widx, 1), :],
                                    start=False,
                                    stop=True,
                                )
                            nc.vector.tensor_copy(
                                ot[:, o0 - s0 : o0 - s0 + ob_], ps[:, :ob_]
                            )
                        t_abs = g0 + i0 + s0
                        nc.scalar.dma_start(
                            out_d[:, t_abs : t_abs + os_, :], ot[:, :os_]
                        )

    nc.compile()
    return nc


def kernel(x=None, cat_ids=None, W=None, b=None, **_unused):
    global last_results
    x = np.asarray(x, np.float32)
    W = np.asarray(W, np.float32)
    N, M, _ = x.shape

    in_maps, scatter, NT, KMAX = _pack(x, cat_ids, W)

    nc = _build(NT, KMAX)

    trace = os.environ.get("CSL_TRACE", "0") == "1"
    kwargs = {}
    if trace:
        kwargs["trace"] = True
        tc_env = os.environ.get("CSL_TRACE_CORES", "")
        if tc_env:
            kwargs["trace_cores"] = [int(c) for c in tc_env.split(",")]
        else:
            kwargs["trace_cores"] = list(range(NCORES))
    res = run_bass_kernel_spmd(
        nc, in_maps, core_ids=list(range(NCORES)), **kwargs
    )
    last_results = res

    out = np.empty((N, M, D_H), np.float32)
    for k in range(NCORES):
        ids, valid = scatter[k]
        # device layout [P, NT, D_H] -> row-major [NT*P, D_H]
        ok = res.results[k]["out"].astype(np.float32, copy=False)
        ok = ok.transpose(1, 0, 2).reshape(NT * SPT, ROWS_PER_SAMPLE, D_H)
        out[ids[valid]] = ok[valid]

    if b is not None:
        b = np.asarray(b, np.float32)
        if np.any(b):
            cat = np.asarray(cat_ids).astype(np.int64).ravel()
            out += b[cat][:, None, :]

    return out



# revision 4
# speedup vs baseline: 1.1346x; 1.1346x over previous
"""Category-specific linear (MoE-routing style) Trainium2 Bass kernel.

Computes out[n] = x[n] @ W[cat_ids[n]] + b[cat_ids[n]] for
x: [N, M, D_IN] f32, cat_ids: [N] int64, W: [C, D_IN, D_H] f32, b: [C, D_H] f32.

Strategy (8-core SPMD, full inputs in / full output out, fully STATIC
device program):
  Host: categories are snake-drafted onto cores by descending size (whole
  categories, optionally pre-split above a size threshold).  All cores share
  one canonical run-length profile: slot r on every core holds canon[r]
  samples (the max over cores at that rank), so run boundaries, weight-slot
  indices and every instruction operand are compile-time constants — no
  dynamic indexing, no TENSOR_LOADs, no per-matmul address patches.  Rows a
  core doesn't fill are zero-padded.  x rows are pre-transposed on the host
  into [2, 128, RT] so the contraction dim lands on SBUF partitions; each
  core gets its own W table [128, 2, R, 256] of just its R categories.
  Device: W is the STATIONARY matmul operand (one [128,128] LDWEIGHTS per
  (run, ic, jc)); x streams as the moving operand in chunks of up to 1024
  rows (bf16 moving max), accumulating over the two 128-deep contraction
  chunks into PSUM.  PSUM->SBUF casts alternate between the Vector and
  Scalar engines (GpSimd has no PSUM port); x loads + out stores ride the
  Sync HWDGE ring while W loads ride the Scalar ring.  Output leaves in
  [2, 128, RT] (D_H-major) layout; the host untransposes and scatters.
"""

import os
import sys

import numpy as np

for _p in ("/opt/trn_rl_repo",):
    if os.path.isdir(_p) and _p not in sys.path:
        sys.path.insert(0, _p)

import concourse.bass as bass  # noqa: E402
import concourse.mybir as mybir  # noqa: E402
import concourse.tile as tile  # noqa: E402
from concourse import bacc  # noqa: E402
from concourse.bass_utils import run_bass_kernel_spmd  # noqa: E402

NCORES = 8
P = 128  # SBUF partitions
D_IN = 256  # contraction dim (2 chunks of 128)
D_H = 256  # output dim (2 chunks of 128)
ROWS_PER_SAMPLE = 16
CHUNK = 512  # max rows per matmul (PSUM out must fit one 2KB f32 bank)
FIRST_CHUNK = 512

# filled by kernel() for test harness introspection
last_results = None


def _snake_profile(sizes_desc):
    """Snake-draft sizes (descending) onto NCORES cores.

    Returns per-core lists of indices into sizes_desc (each list sorted by
    descending size) and the canonical profile canon[r] = max over cores of
    the r-th run size.
    """
    cores = [[] for _ in range(NCORES)]
    for i in range(len(sizes_desc)):
        lap, j = divmod(i, NCORES)
        k = j if lap % 2 == 0 else NCORES - 1 - j
        cores[k].append(i)
    R = max(len(c) for c in cores)
    canon = []
    for r in range(R):
        canon.append(
            max(sizes_desc[c[r]] for c in cores if len(c) > r)
        )
    return cores, canon


def _choose_packing(sizes):
    """Pick a split threshold minimizing total DMA bytes.

    Returns (pieces, cores, canon): pieces is a list of (cat_id, n_samples)
    sorted descending; cores[k] lists piece indices for core k in slot
    order; canon[r] is the canonical samples-per-slot profile.
    """
    present = [(int(s), int(c)) for c, s in enumerate(sizes) if s > 0]
    best = None
    for thresh in (None, 72, 80, 88, 96, 112, 128):
        pieces = []
        for s, c in present:
            if thresh is not None and s > thresh:
                nparts = -(-s // thresh)
                base, rem = divmod(s, nparts)
                for i in range(nparts):
                    pieces.append((base + (1 if i < rem else 0), c))
            else:
                pieces.append((s, c))
        pieces.sort(key=lambda t: -t[0])
        sd = [p[0] for p in pieces]
        cores, canon = _snake_profile(sd)
        # bytes: x load + out store (2B each way) + W table
        cost = (
            2 * sum(canon) * ROWS_PER_SAMPLE * D_H * 2
            + len(canon) * D_IN * D_H * 2
        )
        if best is None or cost < best[0]:
            best = (cost, pieces, cores, canon)
    return best[1], best[2], best[3]


def _np_in_dtype():
    import ml_dtypes

    return {
        "f16": np.float16,
        "bf16": ml_dtypes.bfloat16,
        "f32": np.float32,
    }[_dt_mode()]


def _dt_mode():
    return os.environ.get("CSL_DT_MODE", "bf16")


def _out_mode():
    return os.environ.get("CSL_OUT_DT", "f16")


def _mm_dt():
    return {
        "f16": mybir.dt.float16,
        "bf16": mybir.dt.bfloat16,
        "f32": mybir.dt.float32,
    }[_dt_mode()]


def _pack(x, cat_ids, W):
    """Host-side routing: snake-pack categories, pad to canonical profile,
    transpose x, build per-core weight tables.

    Returns (in_maps, scatter, canon_rows, R) where canon_rows[r] is the
    canonical rows (samples*16) of slot r and scatter[k] = (ids, valid) maps
    canonical sample slots back to original sample indices.
    """
    N, M, Din = x.shape
    assert M == ROWS_PER_SAMPLE and Din == D_IN

    cat = np.asarray(cat_ids).astype(np.int64).ravel()
    C = int(cat.max()) + 1 if len(cat) else 1
    sizes = np.bincount(cat, minlength=C)
    by_cat = {c: np.flatnonzero(cat == c) for c in range(C) if sizes[c]}

    pieces, cores, canon = _choose_packing(sizes)
    R = len(canon)

    # consume each category's sample list piece by piece (pieces of one
    # category are processed in descending-size order; order within the
    # category doesn't matter)
    consumed = {c: 0 for c in by_cat}

    np_in = _np_in_dtype()
    RTs = sum(canon)  # canonical samples per core
    RT = RTs * M  # canonical rows per core

    in_maps = []
    scatter = []
    for k in range(NCORES):
        ids = np.full(RTs, -1, np.int64)
        slot_cats = []
        off = 0
        for r in range(R):
            L = canon[r]
            if r < len(cores[k]):
                n, c = pieces[cores[k][r]]
                lo = consumed[c]
                consumed[c] = lo + n
                ids[off : off + n] = by_cat[c][lo : lo + n]
                slot_cats.append(c)
            else:
                slot_cats.append(pieces[cores[k][0]][1] if cores[k] else 0)
            off += L
        valid = ids >= 0

        Xr = np.zeros((RTs, M, Din), np.float32)
        Xr[valid] = x[ids[valid]]
        xT = np.ascontiguousarray(
            Xr.reshape(RT, Din).T.astype(np_in)
        ).reshape(2, P, RT)

        Wp = W[np.asarray(slot_cats, np.int64)]  # [R, Din, D_H]
        Wl = np.ascontiguousarray(
            Wp.reshape(R, 2, P, D_H).transpose(2, 1, 0, 3).astype(np_in)
        )  # [P, 2, R, D_H]

        in_maps.append({"xT": xT, "Wl": Wl})
        scatter.append((ids, valid))

    canon_rows = tuple(c * M for c in canon)
    return in_maps, scatter, canon_rows, R


def _chunks_of(canon_rows):
    """Static (slot, row_start, row_len) matmul chunks in row order."""
    chunks = []
    off = 0
    for r, L in enumerate(canon_rows):
        pos = 0
        while pos < L:
            step = FIRST_CHUNK if (r == 0 and pos == 0) else CHUNK
            step = min(step, L - pos)
            chunks.append((r, off + pos, step))
            pos += step
        off += L
    return chunks


def _build(canon_rows, R):
    """Build the static SPMD device program."""
    mm_dt = _mm_dt()
    out_dt = mybir.dt.float32 if _out_mode() == "f32" else mybir.dt.float16
    f32 = mybir.dt.float32

    RT = sum(canon_rows)
    chunks = _chunks_of(canon_rows)

    nc = bacc.Bacc(
        "TRN2",
        target_bir_lowering=False,
        debug=False,
        enable_asserts=False,
        num_devices=NCORES,
    )
    xT_d = nc.dram_tensor("xT", [2, P, RT], mm_dt, kind="ExternalInput").ap()
    W_d = nc.dram_tensor("Wl", [P, 2, R, D_H], mm_dt, kind="ExternalInput").ap()
    out_d = nc.dram_tensor("out", [2, P, RT], out_dt, kind="ExternalOutput").ap()

    # x load groups (rows): first small for fast start, then big
    xg = [0, FIRST_CHUNK]
    while xg[-1] < RT:
        xg.append(min(xg[-1] + 2560, RT))

    # store quanta (rows): big in the middle, small at the tail
    sq = [0]
    while RT - sq[-1] > 2560:
        sq.append(sq[-1] + 2048)
    rem = RT - sq[-1]
    if rem > 1024:
        sq.append(sq[-1] + (rem - 1024) // 16 * 16)
        rem = RT - sq[-1]
    if rem > 384:
        sq.append(sq[-1] + (rem - 384) // 16 * 16)
    sq.append(RT)

    with tile.TileContext(nc) as tc:
        with (
            tc.tile_pool(name="wpool", bufs=1) as wpool,
            tc.tile_pool(name="xpool", bufs=1) as xpool,
            tc.tile_pool(name="opool", bufs=1) as opool,
            tc.tile_pool(name="psum", bufs=8, space="PSUM") as psum_pool,
        ):
            W_sb = wpool.tile([P, 2, R, D_H], mm_dt)
            x_sb = xpool.tile([P, 2, RT], mm_dt)
            out_sb = opool.tile([P, 2, RT], out_dt)

            # W slot 0 first (gates the first matmul), then the rest;
            # Scalar HWDGE ring
            nc.scalar.dma_start(W_sb[:, 0, 0:1], W_d[:, 0, 0:1])
            nc.scalar.dma_start(W_sb[:, 1, 0:1], W_d[:, 1, 0:1])
            if R > 1:
                nc.scalar.dma_start(W_sb[:, 0, 1:R], W_d[:, 0, 1:R])
                nc.scalar.dma_start(W_sb[:, 1, 1:R], W_d[:, 1, 1:R])

            # x loads on the Sync HWDGE ring
            for a, bnd in zip(xg, xg[1:]):
                nc.sync.dma_start(x_sb[:, 0, a:bnd], xT_d[0, :, a:bnd])
                nc.sync.dma_start(x_sb[:, 1, a:bnd], xT_d[1, :, a:bnd])

            # greedy cast balancing: DVE 1.04 ns/elem, Act 0.833 ns/elem
            # (Act also spends ~3us issuing W DMAs up front)
            eng_load = {"v": 0.0, "s": 3000.0}

            qi = 0  # next store quantum to emit
            for ci, (r, a, F) in enumerate(chunks):
                for jc in (0, 1):
                    ps = psum_pool.tile([P, CHUNK], f32)
                    nc.tensor.matmul(
                        ps[:, :F],
                        W_sb[:, 0, r, jc * P : (jc + 1) * P],
                        x_sb[:, 0, a : a + F],
                        start=True,
                        stop=False,
                    )
                    nc.tensor.matmul(
                        ps[:, :F],
                        W_sb[:, 1, r, jc * P : (jc + 1) * P],
                        x_sb[:, 1, a : a + F],
                        start=False,
                        stop=True,
                    )
                    if eng_load["v"] <= eng_load["s"]:
                        nc.vector.tensor_copy(out_sb[:, jc, a : a + F], ps[:, :F])
                        eng_load["v"] += F * 1.04
                    else:
                        nc.scalar.activation(
                            out_sb[:, jc, a : a + F],
                            ps[:, :F],
                            mybir.ActivationFunctionType.Copy,
                        )
                        eng_load["s"] += F * 0.833
                # emit stores whose rows are fully produced
                done = a + F
                while qi + 1 < len(sq) and sq[qi + 1] <= done:
                    qa, qb = sq[qi], sq[qi + 1]
                    nc.sync.dma_start(out_d[0, :, qa:qb], out_sb[:, 0, qa:qb])
                    nc.sync.dma_start(out_d[1, :, qa:qb], out_sb[:, 1, qa:qb])
                    qi += 1

    nc.compile()
    return nc


def kernel(x=None, cat_ids=None, W=None, b=None, **_unused):
    global last_results
    x = np.asarray(x, np.float32)
    W = np.asarray(W, np.float32)
    N, M, _ = x.shape

    in_maps, scatter, canon_rows, R = _pack(x, cat_ids, W)

    nc = _build(canon_rows, R)

    trace = os.environ.get("CSL_TRACE", "0") == "1"
    kwargs = {}
    if trace:
        kwargs["trace"] = True
        tc_env = os.environ.get("CSL_TRACE_CORES", "")
        if tc_env:
            kwargs["trace_cores"] = [int(c) for c in tc_env.split(",")]
        else:
            kwargs["trace_cores"] = list(range(NCORES))
    res = run_bass_kernel_spmd(
        nc, in_maps, core_ids=list(range(NCORES)), **kwargs
    )
    last_results = res

    RT = sum(canon_rows)
    RTs = RT // ROWS_PER_SAMPLE
    out = np.empty((N, M, D_H), np.float32)
    for k in range(NCORES):
        ids, valid = scatter[k]
        # device layout [2, P, RT] -> rows [RT, 256] with dh = jc*128 + p
        ok = res.results[k]["out"].astype(np.float32, copy=False)
        ok = ok.transpose(2, 0, 1).reshape(RTs, ROWS_PER_SAMPLE, D_H)
        out[ids[valid]] = ok[valid]

    if b is not None:
        b = np.asarray(b, np.float32)
        if np.any(b):
            cat = np.asarray(cat_ids).astype(np.int64).ravel()
            out += b[cat][:, None, :]

    return out
